# revision 25
# baseline (speedup 1.0000x reference)
"""Distributed Bass kernel for causal multi-head attention with RoPE.

Problem: B=2, S=2048, D=2048, H=16, HD=128 (nn_Attention_85315230368481).

Sharding: head-parallel (tensor-parallel over heads). Core c owns heads
A=2c and B=2c+1. x is replicated (sent transposed to every core).

v2 restructure (vs the 349,874ns v1): heads are processed
HEAD-SEQUENTIALLY so the first AllToAll fires ~85us earlier and the
second one right after attention ends:

  QKV-A | att-A | (a2a#A in flight over QKV-B) | att-B | a2a#B + Oproj(A) | Oproj(B)

 - Each AllToAll costs ~41us in the cost model (15us constant +
   ~1MB/40GBps), so only a2a#B is exposed, and its window is filled
   with the A-head half of the output projection.
 - Normalization is DEFERRED past the collective: the payload carries
   the UNNORMALIZED attention tile (bf16) plus the softmax denominator
   row (AllToAll slices are 129 rows = 128 att + 1 den). The receiver
   does reciprocal (DVE) + partition_broadcast + multiply (Pool) in
   windows where those engines are idle.
 - Softmax denominator: head A accumulates on DVE (bf16 pair-merges at
   2x + f32 accumulate; lenient deadline -- it only gates a2a#A which
   has ~80us of slack and may drain into the QKV-B window). Head B
   splits per tile: (qt0, qt1 both b; qt2 b0) on DVE, (qt3 both b;
   qt2 b1) as PE ones-matmuls into a [1,512] PSUM row (mask column 511
   is all-ones and doubles as the ones vector).
 - Causal diagonal trimming: for the 4 diagonal key-blocks of a query
   tile only queries >= 128*o participate (scores, exp, mask-mul, attV
   and den all use restricted ranges).
 - Host-packed weights: wqkv is one [128, 2*16*3*128] tensor (2 DMAs
   per head phase); wo is pre-block-permuted so Oproj weight tiles are
   straight [128,1024] loads in consumption order.
 - Phase 1 runs dc-major over row-pair chunks so each [128,1024] x
   tile dies after one dc iteration (64 DMA issues per head instead of
   256). Rope uses bf16 temporaries (DVE 2x); Q-rope on DVE, K-rope on
   Pool; swap DMAs on DVE(Q)/Act(K) queues so SP only streams x tiles.

Measured via the timing-only CoreSim cost model (test.py).
"""

import sys

import ml_dtypes
import numpy as np

if "/opt/trn_rl_repo" not in sys.path:
    sys.path.insert(0, "/opt/trn_rl_repo")

B, S, D, H = 2, 2048, 2048, 16
HD = D // H            # 128
NCORES = 8
R = B * S              # 4096 rows (row index = b*S + s)
RC = R // NCORES       # 512 rows per core in the output row-sharding
DCH = D // 128         # 16 contraction chunks
NQT = S // 512         # 4 query tiles of 512 per batch
SCALE = 1.0 / float(np.sqrt(HD))
BF16 = ml_dtypes.bfloat16
SLICE = 129            # a2a slice rows: 128 att + 1 den

_GRAPH = None
_TRACE = False
_LAST_EXEC_NS = None
_LAST_RES = None


def _build_graph():
    import concourse.mybir as mybir
    from concourse import bacc, bass_isa, tile

    f32 = mybir.dt.float32
    bf = mybir.dt.bfloat16
    Exp = mybir.ActivationFunctionType.Exp

    nc = bacc.Bacc("TRN2", target_bir_lowering=False, num_devices=NCORES)

    xT = nc.declare_dram_parameter("xT", [D, R], bf, isOutput=False)
    # wqkv[p, ((lh*16 + dc)*3 + w)*128 + col] = W_w[dc*128+p, head-col]
    wqkv = nc.declare_dram_parameter("wqkv", [128, 2 * DCH * 3 * 128], bf,
                                     isOutput=False)
    # wo[p, (blk*2 + g)*1024 + col]; blk 0..7 = heads 0,2..14 (asb slot
    # order for a2a#A), blk 8..15 = heads 1,3..15.
    wo = nc.declare_dram_parameter("wo", [128, 16 * 2 * 1024], bf,
                                   isOutput=False)
    cos2d = nc.declare_dram_parameter("cos2", [128, S], bf, isOutput=False)
    sin2d = nc.declare_dram_parameter("sin2m", [128, S], bf, isOutput=False)
    bmaskd = nc.declare_dram_parameter("bmask", [128, 4 * 512], bf,
                                       isOutput=False)
    out = nc.declare_dram_parameter("out", [RC, D], f32, isOutput=True)

    HL = 2

    with nc.allow_low_precision(reason="bf16 matmul inputs; fp32 accumulate"), \
         tile.TileContext(nc) as tc:
        with (
            tc.tile_pool(name="dram", bufs=1, space="DRAM") as dramp,
            tc.tile_pool(name="const", bufs=1) as constp,
            tc.tile_pool(name="qk", bufs=1) as qkp,
            tc.tile_pool(name="wqkvp", bufs=2) as wqkvp,
            tc.tile_pool(name="dvacc", bufs=3) as dvacc,
            tc.tile_pool(name="stage", bufs=3) as stage,
            tc.tile_pool(name="normp", bufs=1) as normp,
            tc.tile_pool(name="rbp", bufs=2) as rbp,
            tc.tile_pool(name="rawp", bufs=1) as rawp,
        ):
            a2a_in = [dramp.tile([NCORES * SLICE, RC], bf, name=f"a2a_in{l}")
                      for l in range(HL)]
            a2a_out = [dramp.tile([NCORES * SLICE, RC], bf, name=f"a2a_out{l}")
                       for l in range(HL)]

            cos_sb = constp.tile([128, S], bf, name="cos_sb")
            sin_sb = constp.tile([128, S], bf, name="sin_sb")
            bm_sb = constp.tile([128, 4 * 512], bf, name="bm_sb")

            nc.gpsimd.dma_start(out=cos_sb[:], in_=cos2d[:, :])
            nc.gpsimd.dma_start(out=sin_sb[:], in_=sin2d[:, :])
            nc.gpsimd.dma_start(out=bm_sb[:], in_=bmaskd[:, :])

            def offs(kb, qt):
                return max(kb - 4 * qt, 0)

            # ------------- phase 1: QKV + rope for one head -------------
            def qkv_phase(lh, qsb, ksb, vsb):
                # qsb/ksb: lists of 8 [128,512] tiles (one per row chunk)
                wsb = wqkvp.tile([128, DCH * 384], bf, tag="wsb", name="wsb")
                base = lh * DCH * 3 * 128
                nc.scalar.dma_start(
                    out=wsb[:, 0:2 * 384],
                    in_=wqkv[:, base:base + 2 * 384],
                )
                nc.scalar.dma_start(
                    out=wsb[:, 2 * 384:DCH * 384],
                    in_=wqkv[:, base + 2 * 384:base + DCH * 384],
                )

                def wsl(dc, w):
                    c0 = (dc * 3 + w) * 128
                    return wsb[:, c0:c0 + 128]

                def rope(cp_eng, dma_eng, dst, src_ps, cs, ropep, tg):
                    # rows 0:64 even dims, 64:128 odd dims (host EO-perm)
                    #  dst = tcp*[cos;cos] + swap(tcp)*[-sin;sin]
                    tcp = ropep.tile([128, 512], bf, tag=f"tcp{tg}",
                                     name="tcp")
                    if cp_eng is nc.scalar:
                        nc.scalar.copy(tcp[:], src_ps[:, :])
                    else:
                        cp_eng.tensor_copy(out=tcp[:], in_=src_ps[:, :])
                    tsw = ropep.tile([128, 512], bf, tag=f"tsw{tg}",
                                     name="tsw")
                    dma_eng.dma_start(out=tsw[0:64, :], in_=tcp[64:128, :])
                    dma_eng.dma_start(out=tsw[64:128, :], in_=tcp[0:64, :])
                    t1 = ropep.tile([128, 512], bf, tag=f"rt1{tg}",
                                    name="rt1")
                    t2 = ropep.tile([128, 512], bf, tag=f"rt2{tg}",
                                    name="rt2")
                    nc.vector.tensor_mul(t1[:], tcp[:, :], cos_sb[:, cs])
                    nc.vector.tensor_mul(t2[:], tsw[:, :], sin_sb[:, cs])
                    nc.vector.tensor_add(dst[:, :], t1[:], t2[:])

                with (
                    tc.tile_pool(name=f"xts{lh}", bufs=17) as xtp,
                    tc.tile_pool(name=f"qkps{lh}", bufs=1, space="PSUM") as qkps,
                    tc.tile_pool(name=f"vps{lh}", bufs=1, space="PSUM") as vpsp,
                    tc.tile_pool(name=f"rope{lh}", bufs=4) as ropep,
                ):
                    for pr in range(R // 1024):     # 4 pairs of 512-row chunks
                        last = pr == R // 1024 - 1
                        qps = [qkps.tile([128, 512], f32, tag=f"qps{j}",
                                         name="qps") for j in range(2)]
                        kps = [qkps.tile([128, 512], f32, tag=f"kps{j}",
                                         name="kps") for j in range(2)]
                        vt = vpsp.tile([128, 1024], f32, tag="vt",
                                       name="vt")
                        lastxts = {}
                        for dc in range(DCH):
                            xt = xtp.tile([128, 1024], bf, tag="xt", name="xt")
                            nc.sync.dma_start(
                                out=xt[:],
                                in_=xT[dc * 128:(dc + 1) * 128,
                                       pr * 1024:(pr + 1) * 1024],
                            )
                            if last:
                                lastxts[dc] = xt
                            for j in range(2):
                                xsl = xt[:, j * 512:(j + 1) * 512]
                                nc.tensor.matmul(
                                    qps[j][:], lhsT=wsl(dc, 0), rhs=xsl,
                                    start=(dc == 0), stop=(dc == DCH - 1),
                                )
                                nc.tensor.matmul(
                                    kps[j][:], lhsT=wsl(dc, 1), rhs=xsl,
                                    start=(dc == 0), stop=(dc == DCH - 1),
                                )
                                if not last:
                                    for v4 in range(4):
                                        c0 = (j * 4 + v4) * 128
                                        nc.tensor.matmul(
                                            vt[:, c0:c0 + 128],
                                            lhsT=xt[:, j * 512 + v4 * 128:
                                                    j * 512 + (v4 + 1) * 128],
                                            rhs=wsl(dc, 2),
                                            start=(dc == 0 and v4 == 0),
                                            stop=(dc == DCH - 1 and v4 == 3),
                                        )
                        if last:
                            # V in a second pass over the SAME x tiles (kept
                            # alive by the deep xt ring) so the Q/K ropes of
                            # the final pair overlap the V matmuls and the
                            # att phase starts without a PSUM-WAR stall.
                            for dc in range(DCH):
                                xt = lastxts[dc]
                                for j in range(2):
                                    for v4 in range(4):
                                        c0 = (j * 4 + v4) * 128
                                        nc.tensor.matmul(
                                            vt[:, c0:c0 + 128],
                                            lhsT=xt[:, j * 512 + v4 * 128:
                                                    j * 512 + (v4 + 1) * 128],
                                            rhs=wsl(dc, 2),
                                            start=(dc == 0 and v4 == 0),
                                            stop=(dc == DCH - 1 and v4 == 3),
                                        )
                        for j in range(2):
                            rr = pr * 2 + j
                            sq0 = (rr % 4) * 512
                            cs = slice(sq0, sq0 + 512)
                            rope(nc.scalar, nc.scalar, qsb[rr],
                                 qps[j], cs, ropep, "q")
                            rope(nc.vector, nc.sync, ksb[rr],
                                 kps[j], cs, ropep, "k")
                        for j in range(2):
                            rr = pr * 2 + j
                            nc.vector.tensor_copy(
                                out=vsb[:, rr * 512:(rr + 1) * 512],
                                in_=vt[:, j * 512:(j + 1) * 512])
                        # (ropes of the last pair were emitted above; the
                        # second V pass for the last pair follows in the
                        # loop body above, overlapping the rope drain)

            # ------------- receiver-side normalization -------------
            def recv_load(lh, engines):
                dgs, raw = [], []
                for s in range(NCORES):
                    eng = engines[s % len(engines)]
                    dg = normp.tile([1, 512], bf, tag=f"dgr{s}",
                                    name="dgr")
                    eng.dma_start(
                        out=dg[:],
                        in_=a2a_out[lh][s * SLICE + 128:s * SLICE + 129, :],
                    )
                    dgs.append(dg)
                    t = rawp.tile([128, 512], bf, tag=f"raw{s}",
                                  name="raw")
                    eng.dma_start(
                        out=t[:],
                        in_=a2a_out[lh][s * SLICE:s * SLICE + 128, :],
                    )
                    raw.append(t)
                return raw, dgs

            late = {}

            def recv_norm(raw, dgs, asbcol0, mul_eng):
                asb = late["asb"]
                for s in range(NCORES):
                    rb = rbp.tile([128, 512], bf, tag="rb", name="rb")
                    nc.gpsimd.partition_broadcast(rb[:], dgs[s][0:1, :],
                                                  channels=128)
                    c0 = asbcol0 + s * 512
                    mul_eng.tensor_mul(asb[:, c0:c0 + 512], raw[s][:], rb[:])

            # ------------- phase 2: attention for one head -------------
            # Software-pipelined at tile level: tile T's scores+exp run on
            # PE/Act while tile T-1's attV+den matmuls keep the PE busy, so
            # the PE never waits on an exp chain (stays at full p-state).
            def att_phase(lh, qsb, ksb, vsb, mid_hook=None):
                with (
                    tc.tile_pool(name=f"attps{lh}", bufs=2,
                                 space="PSUM") as attps,
                    tc.tile_pool(name=f"scps{lh}", bufs=2,
                                 space="PSUM") as scps,
                    tc.tile_pool(name=f"denps{lh}", bufs=2,
                                 space="PSUM") as denps,
                    tc.tile_pool(name=f"expool{lh}", bufs=10) as expool,
                    tc.tile_pool(name=f"exsm{lh}", bufs=7) as exsm,
                ):
                    tiles = [(b, qt) for b in range(B)
                             for qt in range(NQT)]
                    st = {}

                    def dve_den(T):
                        # den on the PE everywhere: for den-less tiles the
                        # PE's per-pair work drops below the Act exp rate
                        # and the PE starves; the ones-matmuls are free
                        # relative to those bubbles.
                        return False

                    def emit_scores(T, kp):
                        b, qt = T
                        scp = scps.tile([128, 1024], f32, tag="scp",
                                        name="scp")
                        diag = kp >= 2 * qt
                        for jj in range(2):
                            kb = 2 * kp + jj
                            o = offs(kb, qt)
                            nc.tensor.matmul(
                                scp[:, jj * 512 + o * 128:(jj + 1) * 512],
                                lhsT=ksb[b * 4 + kb // 4][
                                    :, (kb % 4) * 128:(kb % 4 + 1) * 128],
                                rhs=qsb[b * 4 + qt][:, o * 128:512],
                                start=True, stop=True,
                            )
                        ex2 = expool.tile([128, 1024], bf, tag="ex",
                                          name="ex")
                        if not diag:
                            nc.scalar.activation(ex2[:], scp[:], Exp,
                                                 scale=SCALE)
                        else:
                            for jj in range(2):
                                o = offs(2 * kp + jj, qt)
                                sl = slice(jj * 512 + o * 128,
                                           (jj + 1) * 512)
                                nc.scalar.activation(ex2[:, sl], scp[:, sl],
                                                     Exp, scale=SCALE)
                        for jj in range(2):
                            kb = 2 * kp + jj
                            o = offs(kb, qt)
                            w = 512 - o * 128
                            e = ex2[:, jj * 512 + o * 128:(jj + 1) * 512]
                            if kb >= 4 * qt:
                                exm = exsm.tile([128, 512], bf, tag="exm",
                                                name="exm")
                                nc.vector.tensor_mul(
                                    exm[:, 0:w], e,
                                    bm_sb[:, o * 512 + o * 128:
                                          (o + 1) * 512])
                                e = exm[:, 0:w]
                            st[T]["ex"].append((e, kb, o * 128))
                            if dve_den(T):
                                exsum = st[T]["exsum"]
                                if kb == 0:
                                    nc.vector.tensor_copy(out=exsum[:],
                                                          in_=e)
                                else:
                                    nc.vector.tensor_add(
                                        exsum[:, o * 128:512],
                                        exsum[:, o * 128:512], e)

                    def emit_av(T, kp):
                        b, qt = T
                        nkb = 4 * qt + 4
                        for jj in range(2):
                            kb = 2 * kp + jj
                            pex, pkb, pq0 = st[T]["ex"][kb]
                            if not dve_den(T):
                                nc.tensor.matmul(
                                    st[T]["den"][:, pq0:512],
                                    lhsT=bm_sb[:, 511:512], rhs=pex,
                                    start=(pkb == 0), stop=(pkb == nkb - 1),
                                )
                            nc.tensor.matmul(
                                st[T]["att"][:, pq0:512],
                                lhsT=vsb[:, (b * 16 + pkb) * 128:
                                         (b * 16 + pkb + 1) * 128],
                                rhs=pex,
                                start=(pkb == 0), stop=(pkb == nkb - 1),
                            )

                    def emit_payload(T):
                        b, qt = T
                        r = b * 4 + qt
                        atile = stage.tile([128, 512], bf, tag="atile",
                                           name="atile")
                        nc.vector.tensor_copy(out=atile[:],
                                              in_=st[T]["att"][:])
                        nc.sync.dma_start(
                            out=a2a_in[lh][r * SLICE:r * SLICE + 128, :],
                            in_=atile[:],
                        )
                        drow = stage.tile([1, 512], bf, tag="drow",
                                          name="drow")
                        rrow = stage.tile([1, 512], f32, tag="rrow",
                                          name="rrow")
                        if dve_den(T):
                            dred = stage.tile([128, 512], f32, tag="dred",
                                              name="dred")
                            nc.gpsimd.partition_all_reduce(
                                dred[:], st[T]["exsum"][:], channels=128,
                                reduce_op=bass_isa.ReduceOp.add,
                            )
                            nc.vector.reciprocal_approx_fast(
                                rrow[:], dred[0:1, :])
                        else:
                            nc.vector.reciprocal_approx_fast(
                                rrow[:], st[T]["den"][:, :])
                        nc.scalar.copy(drow[:], rrow[:])
                        nc.sync.dma_start(
                            out=a2a_in[lh][r * SLICE + 128:
                                           (r + 1) * SLICE, :],
                            in_=drow[:],
                        )
                        del st[T]

                    # credit-based interleave: keep the scores/exp stream
                    # LEAD pair-units ahead of the attV/den stream so the PE
                    # never waits on an exp chain, including across the small
                    # qt0/qt1 tiles at a phase start.
                    LEAD = 4
                    units = [(T, k) for T in tiles
                             for k in range((4 * T[1] + 4) // 2)]
                    N = len(units)

                    def start_tile(T):
                        st[T] = {
                            "att": attps.tile([128, 512], f32, tag="att",
                                              name="att"),
                            "ex": [],
                        }
                        if dve_den(T):
                            st[T]["exsum"] = dvacc.tile(
                                [128, 512], f32, tag="exsum", name="exsum")
                        else:
                            st[T]["den"] = denps.tile(
                                [1, 512], f32, tag="den", name="den")

                    sc_ptr = 0
                    for av_ptr in range(N):
                        while sc_ptr < N and sc_ptr <= av_ptr + LEAD:
                            T, k = units[sc_ptr]
                            if k == 0:
                                if T == (1, 2) and mid_hook is not None:
                                    mid_hook()
                                start_tile(T)
                            emit_scores(T, k)
                            sc_ptr += 1
                        T, k = units[av_ptr]
                        emit_av(T, k)
                        if k == (4 * T[1] + 4) // 2 - 1:
                            emit_payload(T)

            # ------------- phase 3: output projection halves -------------
            def oproj_half(first, blk0, opsp, drainp, wot_pre):
                asb = late["asb"]
                esb = late["esb"]

                def get_wot(blk, g):
                    key = (blk, g)
                    if key in wot_pre:
                        return wot_pre[key]
                    wt = late["wotp"].tile([128, 1024], bf, tag="wot",
                                           name="wot")
                    nc.sync.dma_start(
                        out=wt[:],
                        in_=wo[:, (blk * 2 + g) * 1024:
                               (blk * 2 + g + 1) * 1024],
                    )
                    wot_pre[key] = wt
                    return wt

                def drain(g, nl, q2, pt):
                    nn = g * 2 + nl
                    es = esb[:, (nn * 4 + q2) * 512:(nn * 4 + q2 + 1) * 512]
                    if first:
                        if q2 % 2 == 0:
                            nc.scalar.copy(es, pt[:])
                        else:
                            nc.vector.tensor_copy(out=es, in_=pt[:])
                    else:
                        osb = drainp.tile([128, 512], f32, tag="osb",
                                          name="osb")
                        nc.vector.tensor_add(osb[:], pt[:], es)
                        oeng = nc.sync if q2 % 2 == 0 else nc.scalar
                        oeng.dma_start(
                            out=out[q2 * 128:(q2 + 1) * 128,
                                    nn * 512:(nn + 1) * 512],
                            in_=osb[:],
                        )

                # group 0: si-outer (streams asb slots as they normalize)
                pt = {}
                for nl in range(2):
                    for q2 in range(4):
                        pt[(nl, q2)] = opsp.tile(
                            [128, 512], f32, tag=f"o{nl}{q2}", name="ops")
                for si in range(8):
                    wt = get_wot(blk0 + si, 0)
                    acol = (blk0 + si) * 512
                    for nl in range(2):
                        for q2 in range(4):
                            nc.tensor.matmul(
                                pt[(nl, q2)][:],
                                lhsT=asb[:, acol + q2 * 128:
                                         acol + (q2 + 1) * 128],
                                rhs=wt[:, nl * 512:(nl + 1) * 512],
                                start=(si == 0), stop=(si == 7),
                            )
                for nl in range(2):
                    for q2 in range(4):
                        drain(0, nl, q2, pt[(nl, q2)])
                # group 1: per-tile si-inner so drains overlap compute and
                # only the last tile's drain is exposed
                for nl in range(2):
                    for q2 in range(4):
                        pt2 = opsp.tile([128, 512], f32, tag=f"o{nl}{q2}",
                                        name="ops")
                        for si in range(8):
                            wt = get_wot(blk0 + si, 1)
                            acol = (blk0 + si) * 512
                            nc.tensor.matmul(
                                pt2[:],
                                lhsT=asb[:, acol + q2 * 128:
                                         acol + (q2 + 1) * 128],
                                rhs=wt[:, nl * 512:(nl + 1) * 512],
                                start=(si == 0), stop=(si == 7),
                            )
                        drain(1, nl, q2, pt2)

            # ========================= schedule =========================
            import concourse.mybir as mybir2

            def a2a(lh):
                nc.gpsimd.collective_compute(
                    "AllToAll",
                    mybir2.AluOpType.bypass,
                    replica_groups=[list(range(NCORES))],
                    ins=[a2a_in[lh].opt()],
                    outs=[a2a_out[lh].opt()],
                )

            # ---- head A ----
            qsbA = [qkp.tile([128, 512], bf, tag=f"qsb{rr}", name="qsbA")
                    for rr in range(8)]
            ksbA = [qkp.tile([128, 512], bf, tag=f"ksb{rr}", name="ksbA")
                    for rr in range(8)]
            vsbA = qkp.tile([128, 32 * 128], bf, tag="vsb", name="vsbA")
            qkv_phase(0, qsbA, ksbA, vsbA)
            att_phase(0, qsbA, ksbA, vsbA)
            a2a(0)

            # ---- head B (a2a#A in flight) ----
            qsbB = [qkp.tile([128, 512], bf, tag=f"qsb{rr}", name="qsbB")
                    for rr in range(8)]
            ksbB = [qkp.tile([128, 512], bf, tag=f"ksb{rr}", name="ksbB")
                    for rr in range(8)]
            vsbB = qkp.tile([128, 32 * 128], bf, tag="vsb", name="vsbB")
            qkv_phase(1, qsbB, ksbB, vsbB)
            with (
                tc.tile_pool(name="latep", bufs=1) as latep,
                tc.tile_pool(name="wotp", bufs=10) as wotp,
            ):
                late["asb"] = latep.tile([128, H * 512], bf, name="asb")
                late["wotp"] = wotp
                with tc.tile_wait_until(0.235):
                    rawA, dgA = recv_load(0, [nc.sync])
                    # wot prefetch for the A-half of Oproj (12 of 16 tiles)
                    wot_pre = {}
                    for blk in range(6):
                        for g in range(2):
                            wt = wotp.tile([128, 1024], bf, tag="wot",
                                           name="wot")
                            nc.sync.dma_start(
                                out=wt[:],
                                in_=wo[:, (blk * 2 + g) * 1024:
                                       (blk * 2 + g + 1) * 1024],
                            )
                            wot_pre[(blk, g)] = wt

                def norm_a():
                    recv_norm(rawA, dgA, 0, nc.gpsimd)

                att_phase(1, qsbB, ksbB, vsbB, mid_hook=norm_a)
                a2a(1)

                # ---- output projection ----
                with (
                    tc.tile_pool(name="esbp", bufs=1) as esbp,
                    tc.tile_pool(name="ops", bufs=1, space="PSUM") as opsp,
                    tc.tile_pool(name="drainp", bufs=4) as drainp,
                ):
                    late["esb"] = esbp.tile([128, 16 * 512], f32,
                                            name="esb")
                    oproj_half(True, 0, opsp, drainp, wot_pre)
                    rawB, dgB = recv_load(1, [nc.scalar, nc.sync])
                    recv_norm(rawB, dgB, 8 * 512, nc.vector)
                    oproj_half(False, 8, opsp, drainp, wot_pre)

    nc.compile()
    return nc


def _get_graph():
    global _GRAPH
    if _GRAPH is None:
        _GRAPH = _build_graph()
    return _GRAPH


# per-head column permutation: even dims then odd dims
_EO = np.concatenate([np.arange(0, HD, 2), np.arange(1, HD, 2)])


def kernel(x, Wq, Wk, Wv, Wo, freqs_cos, freqs_sin, mask):
    global _LAST_EXEC_NS, _LAST_RES
    from concourse.bass_utils import run_bass_kernel_spmd

    nc = _get_graph()

    x = np.asarray(x, np.float32)
    xT = np.ascontiguousarray(x.reshape(R, D).T).astype(BF16)
    cosT_ = np.asarray(freqs_cos, np.float32).T            # [64, S]
    sinT_ = np.asarray(freqs_sin, np.float32).T
    cos2 = np.ascontiguousarray(
        np.concatenate([cosT_, cosT_], axis=0)).astype(BF16)
    sin2m = np.ascontiguousarray(
        np.concatenate([-sinT_, sinT_], axis=0)).astype(BF16)

    # 0/1 relative diagonal masks from the provided additive mask:
    # bm[o][k, q] = 1 iff query q may attend key 128*o + k.
    maskf = np.asarray(mask, np.float32)[0, 0]
    bm = np.empty((128, 4 * 512), np.float32)
    for o in range(4):
        bm[:, o * 512:(o + 1) * 512] = (
            maskf[:512, o * 128:(o + 1) * 128] > -0.5
        ).T.astype(np.float32)
    bm = bm.astype(BF16)

    Wqf = np.asarray(Wq, np.float32)
    Wkf = np.asarray(Wk, np.float32)
    Wvf = np.asarray(Wv, np.float32)
    Wof = np.asarray(Wo, np.float32)

    # wo: [128, (blk*2+g)*1024 + col]; blk order = heads 0,2..14,1,3..15
    blk_heads = list(range(0, H, 2)) + list(range(1, H, 2))
    wo_prep = np.empty((128, 16 * 2 * 1024), np.float32)
    for bi, h in enumerate(blk_heads):
        wo_prep[:, bi * 2048:(bi + 1) * 2048] = Wof[h * 128:(h + 1) * 128, :]
    wo_prep = np.ascontiguousarray(wo_prep).astype(BF16)

    in_maps = []
    for c in range(NCORES):
        wq_h = np.empty((128, 2 * DCH * 3 * 128), np.float32)
        for lh in range(2):
            h = 2 * c + lh
            pq = h * HD + _EO
            nv = np.arange(h * HD, (h + 1) * HD)
            for dc in range(DCH):
                i0 = ((lh * DCH + dc) * 3) * 128
                rows = slice(dc * 128, (dc + 1) * 128)
                wq_h[:, i0:i0 + 128] = Wqf[rows][:, pq]
                wq_h[:, i0 + 128:i0 + 256] = Wkf[rows][:, pq]
                wq_h[:, i0 + 256:i0 + 384] = Wvf[rows][:, nv]
        in_maps.append({
            "xT": xT,
            "wqkv": np.ascontiguousarray(wq_h).astype(BF16),
            "wo": wo_prep,
            "cos2": cos2,
            "sin2m": sin2m,
            "bmask": bm,
        })

    res = run_bass_kernel_spmd(
        nc, in_maps, core_ids=list(range(NCORES)), trace=_TRACE,
    )
    _LAST_EXEC_NS = res.exec_time_ns
    _LAST_RES = res

    outp = np.empty((R, D), np.float32)
    for c in range(NCORES):
        outp[c * RC:(c + 1) * RC, :] = res.results[c]["out"]
    return outp.reshape(B, S, D)
